# revision 24
# baseline (speedup 1.0000x reference)
"""GroupQuantLinear on 8 Trainium2 NeuronCores — fp8 DoubleRow, SBUF-resident W^T.

y[b,s,o] = x[b,s,:] @ W[o,:] + bias[o], where W is dequantized on-device from
4-bit packed weights with per-(o, group) affine scale/bias (groups of 256 along
the 4096-wide input dim).

Sharding: tensor-parallel on out_features (8 shards of 2048 rows); x replicated.

fp8 trick: W = (q - 7.5)*s + (7.5*s + b). The centered nibble value (q - 7.5)
is exactly representable in e4m3, so w_c = 32*(q - 7.5)*s carries only ONE fp8
rounding and about half the dynamic range of W. The main GEMM runs
x_fp8 @ w_c^T in DoubleRow perf mode (2x PE throughput). The affine remainder
is exact: y += t @ (7.5 s + b)^T + bias, with t[bs,g] per-group input sums
computed on host in f64 — folded into the same PSUM accumulation via one tiny
K<=128 bf16 matmul per output tile, closing the accumulation group before the
x(1/32) eviction.

Per-core kernel (Bass/Tile):
  Stage 1 (dequant): stream packed words as uint16 [o-tile 128, 2048 words]
    (2 nibbles per u16 -> only 2 unpack planes at 2x DVE rate), fused DVE
    tensor_scalar (shift+and), then fused DVE tensor_scalar
    (q * (32 s) - 240 s -> fp8) per group. PE-transpose the [o, in'] fp8
    result to [in', o] (PSUM element-step-2) and copy into 4 SBUF-resident
    W^T quarter tiles [128, 32, 512].
  Stage 2 (matmul): custom loop, m-outer (32 tiles of 128 bs rows):
    x fp8 tiles streamed from DRAM once (host pre-cast), k-loop of 16
    DoubleRow slices x n-loop over the 4 resident quarters into 4 live PSUM
    banks; per (m,n) eviction = bias-fold matmul + x(1/32) DVE copy + y DMA.

Host marshalling is layout-only + casts: x^T rows are permuted to the u16
nibble order (in = 2j + plane -> row plane*2048 + j) and cast to e4m3; group
sums t come from the exact f32 x so the remainder path carries no fp8 error.
"""

import numpy as np

B, S, IN, OUT, G = 2, 2048, 4096, 16384, 16
NCORES = 8
OSH = OUT // NCORES       # 2048 out rows per core
BS = B * S                # 4096
NW = IN // 4              # 1024 packed u16 words per out row (4 nibbles each)
P = 128
SC = 32.0                 # w_c pre-scale (exact power of two)
KSUB = IN // P            # 32 k-subtiles
NQ = OSH // 512           # 4 W^T quarters
N_OT = OSH // P           # 16 o-tiles
N_MT = BS // P            # 32 m-tiles
GW = NW // G              # 64 u16 words per group
NWP = NW // P             # 8 in'-tiles per nibble plane

_COMPILED = {}


def _build_nc():
    from contextlib import ExitStack

    import concourse.bass as bass
    import concourse.mybir as mybir
    import concourse.tile as tile
    from concourse import bacc
    from concourse.bass import ds, ts
    from concourse.masks import make_identity

    f32 = mybir.dt.float32
    bf16 = mybir.dt.bfloat16
    fp8 = mybir.dt.float8e4
    u16 = mybir.dt.uint16

    nc = bacc.Bacc(None, target_bir_lowering=False)

    xtp = nc.dram_tensor("xtp", [IN, BS], fp8, kind="ExternalInput")
    wpk = nc.dram_tensor("wpk", [OSH, NW], u16, kind="ExternalInput")
    wsc = nc.dram_tensor("wsc", [OSH, G], f32, kind="ExternalInput")
    wbi = nc.dram_tensor("wbi", [OSH, G], f32, kind="ExternalInput")
    tte = nc.dram_tensor("tte", [P, BS], bf16, kind="ExternalInput")
    be32 = nc.dram_tensor("be32", [P, OSH], bf16, kind="ExternalInput")
    y = nc.dram_tensor("y", [BS, OSH], f32, kind="ExternalOutput")

    with tile.TileContext(nc) as tc:
        with ExitStack() as ctx:
            const = ctx.enter_context(tc.tile_pool(name="const", bufs=1))
            dq = ctx.enter_context(tc.tile_pool(name="dq", bufs=2))
            dq_psum = ctx.enter_context(
                tc.tile_pool(name="dq_psum", bufs=2, space="PSUM")
            )

            tte_sb = const.tile([P, BS], bf16)
            nc.sync.dma_start(tte_sb[:], tte[:])
            be32_sb = const.tile([P, OSH], bf16)
            nc.sync.dma_start(be32_sb[:], be32[:])

            ident = const.tile([P, P], fp8)
            make_identity(nc, ident[:])

            # SBUF-resident W^T quarters: [in-part, ksub, o-chunk] fp8
            wtq = [
                const.tile([P, KSUB, 512], fp8, name=f"wtq{i}") for i in range(NQ)
            ]

            xp = ctx.enter_context(tc.tile_pool(name="xp", bufs=2))
            ev = ctx.enter_context(tc.tile_pool(name="ev", bufs=6))
            mmp = ctx.enter_context(tc.tile_pool(name="mmp", bufs=6, space="PSUM"))

            xv = xtp.rearrange("(ks p) f -> p ks f", p=P)

            # ---- Stage 1 emitter: dequant + transpose one W^T quarter ----
            def emit_quarter(qi):
                for ot in range(4 * qi, 4 * qi + 4):
                    osl = ts(ot, P)
                    t_pk = dq.tile([P, NW], u16, tag="pk", name=f"pk{ot}")
                    nc.sync.dma_start(t_pk[:], wpk[osl, :])
                    t_sc = dq.tile([P, G], f32, tag="sc", name=f"sc{ot}")
                    nc.sync.dma_start(t_sc[:], wsc[osl, :])
                    t_bi = dq.tile([P, G], f32, tag="bi", name=f"bi{ot}")
                    nc.sync.dma_start(t_bi[:], wbi[osl, :])

                    # q4[o, plane, w] = nibble(plane) of word w; in = 4w+plane
                    q4 = dq.tile([P, 4, NW], u16, tag="q4", name=f"q4_{ot}")
                    for k in range(4):
                        nc.vector.tensor_scalar(
                            q4[:, k, :],
                            t_pk[:],
                            4 * k,
                            0xF,
                            mybir.AluOpType.logical_shift_right,
                            mybir.AluOpType.bitwise_and,
                        )
                    # fused dequant q*(32s) + (-240s) -> fp8; group g = w//64
                    wd = dq.tile([P, 4, NW], fp8, tag="wd", name=f"wd{ot}")
                    for g in range(G):
                        nc.vector.tensor_scalar(
                            wd[:, :, ts(g, GW)],
                            q4[:, :, ts(g, GW)],
                            t_sc[:, g : g + 1],
                            t_bi[:, g : g + 1],
                            mybir.AluOpType.mult,
                            mybir.AluOpType.add,
                        )

                    # PE-transpose [o, in'] -> [in', o]; in' = plane*NW + w
                    # fp8 transpose writes PSUM with element step 2.
                    oc = (ot % 4) * P
                    for kb in range(8):  # batches of 4 k-subtiles
                        tps = dq_psum.tile(
                            [P, 4, 2 * P], fp8, tag="tps", name=f"tps{ot}_{kb}"
                        )
                        for s in range(4):
                            it = kb * 4 + s      # ksub = plane*8 + wt
                            nc.tensor.transpose(
                                tps[:, s, ::2],
                                wd[:, it // NWP, ts(it % NWP, P)],
                                ident[:],
                            )
                        nc.vector.tensor_copy(
                            wtq[qi][:, ts(kb, 4), ds(oc, P)], tps[:, :, ::2]
                        )

            # ---- Stage 2 emitter: one n-pair pass over all m-tiles ----
            def emit_cols(ns):
                for mb in range(N_MT // 4):
                    # fetch 4 m-tiles at once: 512B DMA lines
                    xt4 = xp.tile(
                        [P, KSUB, 512], fp8, tag="xt", name=f"xt{ns[0]}_{mb}"
                    )
                    nc.sync.dma_start(xt4[:], xv[:, :, ts(mb, 512)])
                    for mi in range(4):
                        m = 4 * mb + mi
                        msl = ts(m, P)
                        # both fp8 k-loops first, then both bf16 bias folds:
                        # one PE dtype-mode switch pair per m instead of two
                        pss = {}
                        for n in ns:
                            ps = mmp.tile(
                                [P, 512], f32, tag="ps", name=f"ps_{m}_{n}"
                            )
                            pss[n] = ps
                            for k in range(KSUB // 2):
                                kk = ts(k, 2)
                                nc.tensor.matmul(
                                    ps[:],
                                    xt4[:, kk, ts(mi, P)],
                                    wtq[n][:, kk, :],
                                    start=(k == 0),
                                    stop=False,
                                    perf_mode=mybir.MatmulPerfMode.DoubleRow,
                                )
                        for n in ns:
                            # exact affine remainder closes the group
                            nc.tensor.matmul(
                                pss[n][:],
                                tte_sb[:, msl],
                                be32_sb[:, ts(n, 512)],
                                start=False,
                                stop=True,
                                skip_group_check=True,
                            )
                        for n in ns:
                            ot_t = ev.tile(
                                [P, 512], f32, tag="ot", name=f"ot_{m}_{n}"
                            )
                            # eviction on the ACT engine keeps the DVE queue
                            # free for the next quarter's dequant
                            nc.scalar.activation(
                                ot_t[:],
                                pss[n][:],
                                mybir.ActivationFunctionType.Copy,
                                scale=1.0 / SC,
                            )
                            nc.sync.dma_start(y[msl, ts(n, 512)], ot_t[:])

            # interleaved emission: matmul columns start as soon as their
            # W^T quarters exist; later quarters dequantize under the matmuls
            emit_quarter(0)
            emit_quarter(1)
            emit_cols([0, 1])
            emit_quarter(2)
            emit_quarter(3)
            emit_cols([2, 3])

    nc.compile()
    return nc


def _get_compiled():
    if "nc" not in _COMPILED:
        _COMPILED["nc"] = _build_nc()
    return _COMPILED["nc"]


def _marshal(input, w_packed, w_scale, w_bias, bias):
    import ml_dtypes

    x = np.ascontiguousarray(input, dtype=np.float32).reshape(BS, IN)
    # x^T rows permuted plane-major: row plane*1024 + w <- in = 4w + plane
    xt = x.T  # [IN, BS]
    xtp = np.ascontiguousarray(
        xt.reshape(NW, 4, BS).transpose(1, 0, 2).reshape(IN, BS)
    ).astype(ml_dtypes.float8_e4m3)

    # exact per-group input sums (+ ones row), padded to 128 partitions
    t = x.astype(np.float64).reshape(BS, G, IN // G).sum(axis=2)  # [BS, 16]
    tte = np.zeros((P, BS), dtype=ml_dtypes.bfloat16)
    tte[:G, :] = t.T.astype(ml_dtypes.bfloat16)
    tte[G, :] = np.ones(BS, dtype=ml_dtypes.bfloat16)

    s = w_scale.reshape(OUT, G).astype(np.float64)
    b = w_bias.reshape(OUT, G).astype(np.float64)
    be = SC * (7.5 * s + b)  # [OUT, 16]
    brow = SC * bias.reshape(OUT).astype(np.float64)

    wsc2 = (SC * s).astype(np.float32)
    wbi2 = (-7.5 * SC * s).astype(np.float32)

    # only the low u16 half of each int32 word carries nibbles (randint<2^16)
    wpk_u16 = np.ascontiguousarray(
        w_packed.reshape(OUT, NW).view(np.uint16)[:, 0::2]
    )

    in_maps = []
    for c in range(NCORES):
        osl = slice(c * OSH, (c + 1) * OSH)
        be32 = np.zeros((P, OSH), dtype=ml_dtypes.bfloat16)
        be32[:G, :] = be[osl].T.astype(ml_dtypes.bfloat16)
        be32[G, :] = brow[osl].astype(ml_dtypes.bfloat16)
        in_maps.append(
            {
                "xtp": xtp,
                "wpk": np.ascontiguousarray(wpk_u16[osl]),
                "wsc": np.ascontiguousarray(wsc2[osl]),
                "wbi": np.ascontiguousarray(wbi2[osl]),
                "tte": tte,
                "be32": be32,
            }
        )
    return in_maps


def kernel(input, w_packed, w_scale, w_bias, bias, _trace=False, _trace_kwargs=None):
    from concourse.bass_utils import run_bass_kernel_spmd

    nc = _get_compiled()
    in_maps = _marshal(input, w_packed, w_scale, w_bias, bias)
    res = run_bass_kernel_spmd(
        nc,
        in_maps,
        core_ids=list(range(NCORES)),
        trace=_trace,
        **(_trace_kwargs or {}),
    )
    ys = [res.results[c]["y"] for c in range(NCORES)]
    out = np.concatenate(ys, axis=1).reshape(B, S, OUT).astype(np.float32)
    if _trace:
        return out, res
    return out


# revision 25
# speedup vs baseline: 1.1089x; 1.1089x over previous
"""GroupQuantLinear on 8 Trainium2 NeuronCores — fp8 DoubleRow, SBUF-resident W^T.

y[b,s,o] = x[b,s,:] @ W[o,:] + bias[o], where W is dequantized on-device from
4-bit packed weights with per-(o, group) affine scale/bias (groups of 256 along
the 4096-wide input dim).

Sharding: tensor-parallel on out_features (8 shards of 2048 rows); x replicated.

fp8 trick: W = (q - 7.5)*s + (7.5*s + b). The centered nibble value (q - 7.5)
is exactly representable in e4m3, so w_c = 32*(q - 7.5)*s carries only ONE fp8
rounding and about half the dynamic range of W. The main GEMM runs
x_fp8 @ w_c^T in DoubleRow perf mode (2x PE throughput). The affine remainder
is exact: y += t @ (7.5 s + b)^T + bias, with t[bs,g] per-group input sums
computed on host in f64 — folded into the same PSUM accumulation via one tiny
K<=128 bf16 matmul per output tile, closing the accumulation group before the
x(1/32) eviction.

Per-core kernel (Bass/Tile):
  Stage 1 (dequant): stream packed words as uint16 [o-tile 128, 2048 words]
    (2 nibbles per u16 -> only 2 unpack planes at 2x DVE rate), fused DVE
    tensor_scalar (shift+and), then fused DVE tensor_scalar
    (q * (32 s) - 240 s -> fp8) per group. PE-transpose the [o, in'] fp8
    result to [in', o] (PSUM element-step-2) and copy into 4 SBUF-resident
    W^T quarter tiles [128, 32, 512].
  Stage 2 (matmul): custom loop, m-outer (32 tiles of 128 bs rows):
    x fp8 tiles streamed from DRAM once (host pre-cast), k-loop of 16
    DoubleRow slices x n-loop over the 4 resident quarters into 4 live PSUM
    banks; per (m,n) eviction = bias-fold matmul + x(1/32) DVE copy + y DMA.

Host marshalling is layout-only + casts: x^T rows are permuted to the u16
nibble order (in = 2j + plane -> row plane*2048 + j) and cast to e4m3; group
sums t come from the exact f32 x so the remainder path carries no fp8 error.
"""

import numpy as np

B, S, IN, OUT, G = 2, 2048, 4096, 16384, 16
NCORES = 8
OSH = OUT // NCORES       # 2048 out rows per core
BS = B * S                # 4096
NW = IN // 4              # 1024 packed u16 words per out row (4 nibbles each)
P = 128
SC = 32.0                 # w_c pre-scale (exact power of two)
KSUB = IN // P            # 32 k-subtiles
NQ = OSH // 512           # 4 W^T quarters
N_OT = OSH // P           # 16 o-tiles
N_MT = BS // P            # 32 m-tiles
GW = NW // G              # 64 u16 words per group
NWP = NW // P             # 8 in'-tiles per nibble plane

_COMPILED = {}


def _build_nc():
    from contextlib import ExitStack

    import concourse.bass as bass
    import concourse.mybir as mybir
    import concourse.tile as tile
    from concourse import bacc
    from concourse.bass import ds, ts
    from concourse.masks import make_identity

    f32 = mybir.dt.float32
    bf16 = mybir.dt.bfloat16
    fp8 = mybir.dt.float8e4
    u16 = mybir.dt.uint16

    nc = bacc.Bacc(None, target_bir_lowering=False)

    xtp = nc.dram_tensor("xtp", [IN, BS], fp8, kind="ExternalInput")
    wpk = nc.dram_tensor("wpk", [OSH, NW], u16, kind="ExternalInput")
    wsc = nc.dram_tensor("wsc", [OSH, G], f32, kind="ExternalInput")
    wbi = nc.dram_tensor("wbi", [OSH, G], f32, kind="ExternalInput")
    tte = nc.dram_tensor("tte", [P, BS], bf16, kind="ExternalInput")
    be32 = nc.dram_tensor("be32", [P, OSH], bf16, kind="ExternalInput")
    y = nc.dram_tensor("y", [BS, OSH], f32, kind="ExternalOutput")

    with tile.TileContext(nc) as tc:
        with ExitStack() as ctx:
            const = ctx.enter_context(tc.tile_pool(name="const", bufs=1))
            dq = ctx.enter_context(tc.tile_pool(name="dq", bufs=2))
            dq_psum = ctx.enter_context(
                tc.tile_pool(name="dq_psum", bufs=2, space="PSUM")
            )

            tte_sb = const.tile([P, BS], bf16)
            nc.sync.dma_start(tte_sb[:], tte[:])
            be32_sb = const.tile([P, OSH], bf16)
            nc.sync.dma_start(be32_sb[:], be32[:])

            ident = const.tile([P, P], fp8)
            make_identity(nc, ident[:])

            # SBUF-resident W^T quarters: [in-part, ksub, o-chunk] fp8
            wtq = [
                const.tile([P, KSUB, 512], fp8, name=f"wtq{i}") for i in range(NQ)
            ]

            xp = ctx.enter_context(tc.tile_pool(name="xp", bufs=2))
            ev = ctx.enter_context(tc.tile_pool(name="ev", bufs=6))
            mmp = ctx.enter_context(tc.tile_pool(name="mmp", bufs=6, space="PSUM"))

            xv = xtp.rearrange("(ks p) f -> p ks f", p=P)

            # ---- Stage 1 emitter: dequant + transpose one W^T quarter ----
            def emit_quarter(qi):
                for ot in range(4 * qi, 4 * qi + 4):
                    osl = ts(ot, P)
                    t_pk = dq.tile([P, NW], u16, tag="pk", name=f"pk{ot}")
                    nc.sync.dma_start(t_pk[:], wpk[osl, :])
                    t_sc = dq.tile([P, G], f32, tag="sc", name=f"sc{ot}")
                    nc.sync.dma_start(t_sc[:], wsc[osl, :])
                    t_bi = dq.tile([P, G], f32, tag="bi", name=f"bi{ot}")
                    nc.sync.dma_start(t_bi[:], wbi[osl, :])

                    # q4[o, plane, w] = nibble(plane) of word w; in = 4w+plane
                    q4 = dq.tile([P, 4, NW], u16, tag="q4", name=f"q4_{ot}")
                    for k in range(4):
                        nc.vector.tensor_scalar(
                            q4[:, k, :],
                            t_pk[:],
                            4 * k,
                            0xF,
                            mybir.AluOpType.logical_shift_right,
                            mybir.AluOpType.bitwise_and,
                        )
                    # fused dequant q*(32s) + (-240s) -> fp8; group g = w//64
                    wd = dq.tile([P, 4, NW], fp8, tag="wd", name=f"wd{ot}")
                    for g in range(G):
                        nc.vector.tensor_scalar(
                            wd[:, :, ts(g, GW)],
                            q4[:, :, ts(g, GW)],
                            t_sc[:, g : g + 1],
                            t_bi[:, g : g + 1],
                            mybir.AluOpType.mult,
                            mybir.AluOpType.add,
                        )

                    # PE-transpose [o, in'] -> [in', o]; in' = plane*NW + w
                    # fp8 transpose writes PSUM with element step 2.
                    oc = (ot % 4) * P
                    for kb in range(8):  # batches of 4 k-subtiles
                        tps = dq_psum.tile(
                            [P, 4, 2 * P], fp8, tag="tps", name=f"tps{ot}_{kb}"
                        )
                        for s in range(4):
                            it = kb * 4 + s      # ksub = plane*8 + wt
                            nc.tensor.transpose(
                                tps[:, s, ::2],
                                wd[:, it // NWP, ts(it % NWP, P)],
                                ident[:],
                            )
                        nc.any.tensor_copy(
                            wtq[qi][:, ts(kb, 4), ds(oc, P)], tps[:, :, ::2]
                        )

            # ---- Stage 2 emitter: one n-pair pass over all m-tiles ----
            def emit_cols(ns):
                for mb in range(N_MT // 4):
                    # fetch 4 m-tiles at once: 512B DMA lines
                    xt4 = xp.tile(
                        [P, KSUB, 512], fp8, tag="xt", name=f"xt{ns[0]}_{mb}"
                    )
                    nc.sync.dma_start(xt4[:], xv[:, :, ts(mb, 512)])
                    for mi in range(4):
                        m = 4 * mb + mi
                        msl = ts(m, P)
                        # both fp8 k-loops first, then both bf16 bias folds:
                        # one PE dtype-mode switch pair per m instead of two
                        pss = {}
                        for n in ns:
                            ps = mmp.tile(
                                [P, 512], f32, tag="ps", name=f"ps_{m}_{n}"
                            )
                            pss[n] = ps
                            for k in range(KSUB // 2):
                                kk = ts(k, 2)
                                nc.tensor.matmul(
                                    ps[:],
                                    xt4[:, kk, ts(mi, P)],
                                    wtq[n][:, kk, :],
                                    start=(k == 0),
                                    stop=False,
                                    perf_mode=mybir.MatmulPerfMode.DoubleRow,
                                )
                        for n in ns:
                            # exact affine remainder closes the group
                            nc.tensor.matmul(
                                pss[n][:],
                                tte_sb[:, msl],
                                be32_sb[:, ts(n, 512)],
                                start=False,
                                stop=True,
                                skip_group_check=True,
                            )
                        for n in ns:
                            ot_t = ev.tile(
                                [P, 512], f32, tag="ot", name=f"ot_{m}_{n}"
                            )
                            # eviction on the ACT engine keeps the DVE queue
                            # free for the next quarter's dequant
                            nc.scalar.activation(
                                ot_t[:],
                                pss[n][:],
                                mybir.ActivationFunctionType.Copy,
                                scale=1.0 / SC,
                            )
                            nc.sync.dma_start(y[msl, ts(n, 512)], ot_t[:])

            # interleaved emission: matmul columns start as soon as their
            # W^T quarters exist; later quarters dequantize under the matmuls
            emit_quarter(0)
            emit_quarter(1)
            emit_cols([0, 1])
            emit_quarter(2)
            emit_quarter(3)
            emit_cols([2, 3])

    nc.compile()
    return nc


def _get_compiled():
    if "nc" not in _COMPILED:
        _COMPILED["nc"] = _build_nc()
    return _COMPILED["nc"]


def _marshal(input, w_packed, w_scale, w_bias, bias):
    import ml_dtypes

    x = np.ascontiguousarray(input, dtype=np.float32).reshape(BS, IN)
    # x^T rows permuted plane-major: row plane*1024 + w <- in = 4w + plane
    xt = x.T  # [IN, BS]
    xtp = np.ascontiguousarray(
        xt.reshape(NW, 4, BS).transpose(1, 0, 2).reshape(IN, BS)
    ).astype(ml_dtypes.float8_e4m3)

    # exact per-group input sums (+ ones row), padded to 128 partitions
    t = x.astype(np.float64).reshape(BS, G, IN // G).sum(axis=2)  # [BS, 16]
    tte = np.zeros((P, BS), dtype=ml_dtypes.bfloat16)
    tte[:G, :] = t.T.astype(ml_dtypes.bfloat16)
    tte[G, :] = np.ones(BS, dtype=ml_dtypes.bfloat16)

    s = w_scale.reshape(OUT, G).astype(np.float64)
    b = w_bias.reshape(OUT, G).astype(np.float64)
    be = SC * (7.5 * s + b)  # [OUT, 16]
    brow = SC * bias.reshape(OUT).astype(np.float64)

    wsc2 = (SC * s).astype(np.float32)
    wbi2 = (-7.5 * SC * s).astype(np.float32)

    # only the low u16 half of each int32 word carries nibbles (randint<2^16)
    wpk_u16 = np.ascontiguousarray(
        w_packed.reshape(OUT, NW).view(np.uint16)[:, 0::2]
    )

    in_maps = []
    for c in range(NCORES):
        osl = slice(c * OSH, (c + 1) * OSH)
        be32 = np.zeros((P, OSH), dtype=ml_dtypes.bfloat16)
        be32[:G, :] = be[osl].T.astype(ml_dtypes.bfloat16)
        be32[G, :] = brow[osl].astype(ml_dtypes.bfloat16)
        in_maps.append(
            {
                "xtp": xtp,
                "wpk": np.ascontiguousarray(wpk_u16[osl]),
                "wsc": np.ascontiguousarray(wsc2[osl]),
                "wbi": np.ascontiguousarray(wbi2[osl]),
                "tte": tte,
                "be32": be32,
            }
        )
    return in_maps


def kernel(input, w_packed, w_scale, w_bias, bias, _trace=False, _trace_kwargs=None):
    from concourse.bass_utils import run_bass_kernel_spmd

    nc = _get_compiled()
    in_maps = _marshal(input, w_packed, w_scale, w_bias, bias)
    res = run_bass_kernel_spmd(
        nc,
        in_maps,
        core_ids=list(range(NCORES)),
        trace=_trace,
        **(_trace_kwargs or {}),
    )
    ys = [res.results[c]["y"] for c in range(NCORES)]
    out = np.concatenate(ys, axis=1).reshape(B, S, OUT).astype(np.float32)
    if _trace:
        return out, res
    return out


# revision 26
# speedup vs baseline: 1.1301x; 1.0191x over previous
"""GroupQuantLinear on 8 Trainium2 NeuronCores — fp8 DoubleRow, SBUF-resident W^T.

y[b,s,o] = x[b,s,:] @ W[o,:] + bias[o], where W is dequantized on-device from
4-bit packed weights with per-(o, group) affine scale/bias (groups of 256 along
the 4096-wide input dim).

Sharding: tensor-parallel on out_features (8 shards of 2048 rows); x replicated.

fp8 trick: W = (q - 7.5)*s + (7.5*s + b). The centered nibble value (q - 7.5)
is exactly representable in e4m3, so w_c = 32*(q - 7.5)*s carries only ONE fp8
rounding and about half the dynamic range of W. The main GEMM runs
x_fp8 @ w_c^T in DoubleRow perf mode (2x PE throughput). The affine remainder
is exact: y += t @ (7.5 s + b)^T + bias, with t[bs,g] per-group input sums
computed on host in f64 — folded into the same PSUM accumulation via one tiny
K<=128 bf16 matmul per output tile, closing the accumulation group before the
x(1/32) eviction.

Per-core kernel (Bass/Tile):
  Stage 1 (dequant): stream packed words as uint16 [o-tile 128, 2048 words]
    (2 nibbles per u16 -> only 2 unpack planes at 2x DVE rate), fused DVE
    tensor_scalar (shift+and), then fused DVE tensor_scalar
    (q * (32 s) - 240 s -> fp8) per group. PE-transpose the [o, in'] fp8
    result to [in', o] (PSUM element-step-2) and copy into 4 SBUF-resident
    W^T quarter tiles [128, 32, 512].
  Stage 2 (matmul): custom loop, m-outer (32 tiles of 128 bs rows):
    x fp8 tiles streamed from DRAM once (host pre-cast), k-loop of 16
    DoubleRow slices x n-loop over the 4 resident quarters into 4 live PSUM
    banks; per (m,n) eviction = bias-fold matmul + x(1/32) DVE copy + y DMA.

Host marshalling is layout-only + casts: x^T rows are permuted to the u16
nibble order (in = 2j + plane -> row plane*2048 + j) and cast to e4m3; group
sums t come from the exact f32 x so the remainder path carries no fp8 error.
"""

import numpy as np

B, S, IN, OUT, G = 2, 2048, 4096, 16384, 16
NCORES = 8
OSH = OUT // NCORES       # 2048 out rows per core
BS = B * S                # 4096
NW = IN // 4              # 1024 packed u16 words per out row (4 nibbles each)
P = 128
SC = 32.0                 # w_c pre-scale (exact power of two)
KSUB = IN // P            # 32 k-subtiles
NQ = OSH // 512           # 4 W^T quarters
N_OT = OSH // P           # 16 o-tiles
N_MT = BS // P            # 32 m-tiles
GW = NW // G              # 64 u16 words per group
NWP = NW // P             # 8 in'-tiles per nibble plane

_COMPILED = {}


def _build_nc():
    from contextlib import ExitStack

    import concourse.bass as bass
    import concourse.mybir as mybir
    import concourse.tile as tile
    from concourse import bacc
    from concourse.bass import ds, ts
    from concourse.masks import make_identity

    f32 = mybir.dt.float32
    bf16 = mybir.dt.bfloat16
    fp8 = mybir.dt.float8e4
    u16 = mybir.dt.uint16

    nc = bacc.Bacc(None, target_bir_lowering=False)

    xtp = nc.dram_tensor("xtp", [IN, BS], fp8, kind="ExternalInput")
    wpk = nc.dram_tensor("wpk", [OSH, NW], u16, kind="ExternalInput")
    wsc = nc.dram_tensor("wsc", [OSH, G], f32, kind="ExternalInput")
    wbi = nc.dram_tensor("wbi", [OSH, G], f32, kind="ExternalInput")
    tte = nc.dram_tensor("tte", [P, BS], bf16, kind="ExternalInput")
    be32 = nc.dram_tensor("be32", [P, OSH], bf16, kind="ExternalInput")
    y = nc.dram_tensor("y", [BS, OSH], f32, kind="ExternalOutput")

    with tile.TileContext(nc) as tc:
        with ExitStack() as ctx:
            const = ctx.enter_context(tc.tile_pool(name="const", bufs=1))
            dq = ctx.enter_context(tc.tile_pool(name="dq", bufs=3))
            dq_psum = ctx.enter_context(
                tc.tile_pool(name="dq_psum", bufs=3, space="PSUM")
            )

            tte_sb = const.tile([P, BS], bf16)
            nc.sync.dma_start(tte_sb[:], tte[:])
            be32_sb = const.tile([P, OSH], bf16)
            nc.sync.dma_start(be32_sb[:], be32[:])

            ident = const.tile([P, P], fp8)
            make_identity(nc, ident[:])

            # SBUF-resident W^T quarters: [in-part, ksub, o-chunk] fp8
            wtq = [
                const.tile([P, KSUB, 512], fp8, name=f"wtq{i}") for i in range(NQ)
            ]

            xp = ctx.enter_context(tc.tile_pool(name="xp", bufs=2))
            ev = ctx.enter_context(tc.tile_pool(name="ev", bufs=8))
            mmp = ctx.enter_context(tc.tile_pool(name="mmp", bufs=5, space="PSUM"))

            xv = xtp.rearrange("(ks p) f -> p ks f", p=P)

            # ---- Stage 1 emitter: dequant + transpose one W^T quarter ----
            def emit_quarter(qi):
                for ot in range(4 * qi, 4 * qi + 4):
                    osl = ts(ot, P)
                    t_pk = dq.tile([P, NW], u16, tag="pk", name=f"pk{ot}")
                    nc.sync.dma_start(t_pk[:], wpk[osl, :])
                    t_sc = dq.tile([P, G], f32, tag="sc", name=f"sc{ot}")
                    nc.sync.dma_start(t_sc[:], wsc[osl, :])
                    t_bi = dq.tile([P, G], f32, tag="bi", name=f"bi{ot}")
                    nc.sync.dma_start(t_bi[:], wbi[osl, :])

                    # q4[o, plane, w] = nibble(plane) of word w; in = 4w+plane
                    q4 = dq.tile([P, 4, NW], u16, tag="q4", name=f"q4_{ot}")
                    for k in range(4):
                        nc.vector.tensor_scalar(
                            q4[:, k, :],
                            t_pk[:],
                            4 * k,
                            0xF,
                            mybir.AluOpType.logical_shift_right,
                            mybir.AluOpType.bitwise_and,
                        )
                    # fused dequant q*(32s) + (-240s) -> fp8; group g = w//64
                    wd = dq.tile([P, 4, NW], fp8, tag="wd", name=f"wd{ot}")
                    for g in range(G):
                        nc.vector.tensor_scalar(
                            wd[:, :, ts(g, GW)],
                            q4[:, :, ts(g, GW)],
                            t_sc[:, g : g + 1],
                            t_bi[:, g : g + 1],
                            mybir.AluOpType.mult,
                            mybir.AluOpType.add,
                        )

                    # PE-transpose [o, in'] -> [in', o]; in' = plane*NW + w
                    # fp8 transpose writes PSUM with element step 2.
                    oc = (ot % 4) * P
                    for kb in range(8):  # batches of 4 k-subtiles
                        tps = dq_psum.tile(
                            [P, 4, 2 * P], fp8, tag="tps", name=f"tps{ot}_{kb}"
                        )
                        for s in range(4):
                            it = kb * 4 + s      # ksub = plane*8 + wt
                            nc.tensor.transpose(
                                tps[:, s, ::2],
                                wd[:, it // NWP, ts(it % NWP, P)],
                                ident[:],
                            )
                        nc.any.tensor_copy(
                            wtq[qi][:, ts(kb, 4), ds(oc, P)], tps[:, :, ::2]
                        )

            # ---- Stage 2 emitter: one n-pair pass over all m-tiles ----
            def emit_cols(ns):
                for mb in range(N_MT // 4):
                    # fetch 4 m-tiles at once: 512B DMA lines
                    xt4 = xp.tile(
                        [P, KSUB, 512], fp8, tag="xt", name=f"xt{ns[0]}_{mb}"
                    )
                    nc.sync.dma_start(xt4[:], xv[:, :, ts(mb, 512)])
                    for mi in range(4):
                        m = 4 * mb + mi
                        msl = ts(m, P)
                        # both fp8 k-loops first, then both bf16 bias folds:
                        # one PE dtype-mode switch pair per m instead of two
                        pss = {}
                        for n in ns:
                            ps = mmp.tile(
                                [P, 512], f32, tag="ps", name=f"ps_{m}_{n}"
                            )
                            pss[n] = ps
                            for k in range(KSUB // 2):
                                kk = ts(k, 2)
                                nc.tensor.matmul(
                                    ps[:],
                                    xt4[:, kk, ts(mi, P)],
                                    wtq[n][:, kk, :],
                                    start=(k == 0),
                                    stop=False,
                                    perf_mode=mybir.MatmulPerfMode.DoubleRow,
                                )
                        for n in ns:
                            # exact affine remainder closes the group
                            nc.tensor.matmul(
                                pss[n][:],
                                tte_sb[:, msl],
                                be32_sb[:, ts(n, 512)],
                                start=False,
                                stop=True,
                                skip_group_check=True,
                            )
                        for n in ns:
                            ot_t = ev.tile(
                                [P, 512], f32, tag="ot", name=f"ot_{m}_{n}"
                            )
                            # eviction on the ACT engine keeps the DVE queue
                            # free for the next quarter's dequant
                            nc.scalar.activation(
                                ot_t[:],
                                pss[n][:],
                                mybir.ActivationFunctionType.Copy,
                                scale=1.0 / SC,
                            )
                            nc.sync.dma_start(y[msl, ts(n, 512)], ot_t[:])

            # interleaved emission: matmul columns start as soon as their
            # W^T quarters exist; later quarters dequantize under the matmuls
            emit_quarter(0)
            emit_quarter(1)
            emit_cols([0, 1])
            emit_quarter(2)
            emit_quarter(3)
            emit_cols([2, 3])

    nc.compile()
    return nc


def _get_compiled():
    if "nc" not in _COMPILED:
        _COMPILED["nc"] = _build_nc()
    return _COMPILED["nc"]


def _marshal(input, w_packed, w_scale, w_bias, bias):
    import ml_dtypes

    x = np.ascontiguousarray(input, dtype=np.float32).reshape(BS, IN)
    # x^T rows permuted plane-major: row plane*1024 + w <- in = 4w + plane
    xt = x.T  # [IN, BS]
    xtp = np.ascontiguousarray(
        xt.reshape(NW, 4, BS).transpose(1, 0, 2).reshape(IN, BS)
    ).astype(ml_dtypes.float8_e4m3)

    # exact per-group input sums (+ ones row), padded to 128 partitions
    t = x.astype(np.float64).reshape(BS, G, IN // G).sum(axis=2)  # [BS, 16]
    tte = np.zeros((P, BS), dtype=ml_dtypes.bfloat16)
    tte[:G, :] = t.T.astype(ml_dtypes.bfloat16)
    tte[G, :] = np.ones(BS, dtype=ml_dtypes.bfloat16)

    s = w_scale.reshape(OUT, G).astype(np.float64)
    b = w_bias.reshape(OUT, G).astype(np.float64)
    be = SC * (7.5 * s + b)  # [OUT, 16]
    brow = SC * bias.reshape(OUT).astype(np.float64)

    wsc2 = (SC * s).astype(np.float32)
    wbi2 = (-7.5 * SC * s).astype(np.float32)

    # only the low u16 half of each int32 word carries nibbles (randint<2^16)
    wpk_u16 = np.ascontiguousarray(
        w_packed.reshape(OUT, NW).view(np.uint16)[:, 0::2]
    )

    in_maps = []
    for c in range(NCORES):
        osl = slice(c * OSH, (c + 1) * OSH)
        be32 = np.zeros((P, OSH), dtype=ml_dtypes.bfloat16)
        be32[:G, :] = be[osl].T.astype(ml_dtypes.bfloat16)
        be32[G, :] = brow[osl].astype(ml_dtypes.bfloat16)
        in_maps.append(
            {
                "xtp": xtp,
                "wpk": np.ascontiguousarray(wpk_u16[osl]),
                "wsc": np.ascontiguousarray(wsc2[osl]),
                "wbi": np.ascontiguousarray(wbi2[osl]),
                "tte": tte,
                "be32": be32,
            }
        )
    return in_maps


def kernel(input, w_packed, w_scale, w_bias, bias, _trace=False, _trace_kwargs=None):
    from concourse.bass_utils import run_bass_kernel_spmd

    nc = _get_compiled()
    in_maps = _marshal(input, w_packed, w_scale, w_bias, bias)
    res = run_bass_kernel_spmd(
        nc,
        in_maps,
        core_ids=list(range(NCORES)),
        trace=_trace,
        **(_trace_kwargs or {}),
    )
    ys = [res.results[c]["y"] for c in range(NCORES)]
    out = np.concatenate(ys, axis=1).reshape(B, S, OUT).astype(np.float32)
    if _trace:
        return out, res
    return out


# revision 30
# speedup vs baseline: 1.1608x; 1.0272x over previous
"""GroupQuantLinear on 8 Trainium2 NeuronCores — fp8 DoubleRow, SBUF-resident W^T.

y[b,s,o] = x[b,s,:] @ W[o,:] + bias[o], where W is dequantized on-device from
4-bit packed weights with per-(o, group) affine scale/bias (groups of 256 along
the 4096-wide input dim).

Sharding: tensor-parallel on out_features (8 shards of 2048 rows); x replicated.

fp8 trick: W = (q - 7.5)*s + (7.5*s + b). The centered nibble value (q - 7.5)
is exactly representable in e4m3, so w_c = 32*(q - 7.5)*s carries only ONE fp8
rounding and about half the dynamic range of W. The main GEMM runs
x_fp8 @ w_c^T in DoubleRow perf mode (2x PE throughput). The affine remainder
is exact: y += t @ (7.5 s + b)^T + bias, with t[bs,g] per-group input sums
computed on host in f64 — folded into the same PSUM accumulation via one tiny
K<=128 bf16 matmul per output tile, closing the accumulation group before the
x(1/32) eviction.

Per-core kernel (Bass/Tile):
  Stage 1 (dequant): stream packed words as uint16 [o-tile 128, 1024 words]
    (all 4 nibbles sit in the low half of each int32, so the u16 view halves
    the DMA and doubles DVE rate), one fused DVE tensor_scalar (shift+and)
    per nibble plane, then one fused DVE tensor_scalar
    (q * (32 s) - 240 s -> fp8) per group. PE-transpose the [o, in'] fp8
    result to [in', o] (fp8 transposes write PSUM with element step 2) and
    copy into 4 SBUF-resident W^T quarter tiles [128, 32, 512] — no DRAM
    round-trip for W^T, and stage 2 never re-streams it.
  Stage 2 (matmul): custom loop emitted in interleaved order
    (quarters 0,1 -> columns 0,1 over all m -> quarters 2,3 -> columns 2,3)
    so later quarters dequantize underneath the running matmuls. Per m-tile
    of 128 rows: 2x16 DoubleRow k-loop into 2 PSUM banks, both bf16
    bias-fold matmuls, then x(1/32) evictions on the ACT engine + y DMA.

Host marshalling is layout-only + casts: x^T rows are permuted plane-major
(in = 4w + plane -> row plane*1024 + w) and cast to e4m3; group sums t come
from the exact f32 x so the remainder path carries no fp8 error.
"""

import numpy as np

B, S, IN, OUT, G = 2, 2048, 4096, 16384, 16
NCORES = 8
OSH = OUT // NCORES       # 2048 out rows per core
BS = B * S                # 4096
NW = IN // 4              # 1024 packed u16 words per out row (4 nibbles each)
P = 128
SC = 32.0                 # w_c pre-scale (exact power of two)
KSUB = IN // P            # 32 k-subtiles
NQ = OSH // 512           # 4 W^T quarters
N_OT = OSH // P           # 16 o-tiles
N_MT = BS // P            # 32 m-tiles
GW = NW // G              # 64 u16 words per group
NWP = NW // P             # 8 in'-tiles per nibble plane

_COMPILED = {}


def _build_nc():
    from contextlib import ExitStack

    import concourse.bass as bass
    import concourse.mybir as mybir
    import concourse.tile as tile
    from concourse import bacc
    from concourse.bass import ds, ts

    f32 = mybir.dt.float32
    bf16 = mybir.dt.bfloat16
    fp8 = mybir.dt.float8e4
    u16 = mybir.dt.uint16

    nc = bacc.Bacc(None, target_bir_lowering=False)

    xtp = nc.dram_tensor("xtp", [IN, BS], fp8, kind="ExternalInput")
    wpk = nc.dram_tensor("wpk", [OSH, NW], u16, kind="ExternalInput")
    wsc = nc.dram_tensor("wsc", [OSH, G], f32, kind="ExternalInput")
    wbi = nc.dram_tensor("wbi", [OSH, G], f32, kind="ExternalInput")
    tte = nc.dram_tensor("tte", [P, BS], bf16, kind="ExternalInput")
    id8 = nc.dram_tensor("id8", [P, P], fp8, kind="ExternalInput")
    be32 = nc.dram_tensor("be32", [P, OSH], bf16, kind="ExternalInput")
    y = nc.dram_tensor("y", [BS, OSH], f32, kind="ExternalOutput")

    with tile.TileContext(nc) as tc:
        with ExitStack() as ctx:
            const = ctx.enter_context(tc.tile_pool(name="const", bufs=1))
            dq = ctx.enter_context(tc.tile_pool(name="dq", bufs=3))
            dq_psum = ctx.enter_context(
                tc.tile_pool(name="dq_psum", bufs=3, space="PSUM")
            )

            tte_sb = const.tile([P, BS], bf16)
            nc.sync.dma_start(tte_sb[:], tte[:])
            be32_sb = const.tile([P, OSH], bf16)
            nc.sync.dma_start(be32_sb[:], be32[:])

            ident = const.tile([P, P], fp8)
            nc.sync.dma_start(ident[:], id8[:])

            # SBUF-resident W^T quarters: [in-part, ksub, o-chunk] fp8
            wtq = [
                const.tile([P, KSUB, 512], fp8, name=f"wtq{i}") for i in range(NQ)
            ]

            xp = ctx.enter_context(tc.tile_pool(name="xp", bufs=2))
            ev = ctx.enter_context(tc.tile_pool(name="ev", bufs=8))
            mmp = ctx.enter_context(tc.tile_pool(name="mmp", bufs=5, space="PSUM"))

            xv = xtp.rearrange("(ks p) f -> p ks f", p=P)

            # ---- Stage 1 emitter: dequant + transpose one W^T quarter ----
            def emit_quarter(qi):
                for ot in range(4 * qi, 4 * qi + 4):
                    osl = ts(ot, P)
                    t_pk = dq.tile([P, NW], u16, tag="pk", name=f"pk{ot}")
                    nc.sync.dma_start(t_pk[:], wpk[osl, :])
                    t_sc = dq.tile([P, G], f32, tag="sc", name=f"sc{ot}")
                    nc.sync.dma_start(t_sc[:], wsc[osl, :])
                    t_bi = dq.tile([P, G], f32, tag="bi", name=f"bi{ot}")
                    nc.sync.dma_start(t_bi[:], wbi[osl, :])

                    # q4[o, plane, w] = nibble(plane) of word w; in = 4w+plane
                    q4 = dq.tile([P, 4, NW], u16, tag="q4", name=f"q4_{ot}")
                    for k in range(4):
                        nc.vector.tensor_scalar(
                            q4[:, k, :],
                            t_pk[:],
                            4 * k,
                            0xF,
                            mybir.AluOpType.logical_shift_right,
                            mybir.AluOpType.bitwise_and,
                        )
                    # fused dequant q*(32s) + (-240s) -> fp8; group g = w//64
                    wd = dq.tile([P, 4, NW], fp8, tag="wd", name=f"wd{ot}")
                    for g in range(G):
                        nc.vector.tensor_scalar(
                            wd[:, :, ts(g, GW)],
                            q4[:, :, ts(g, GW)],
                            t_sc[:, g : g + 1],
                            t_bi[:, g : g + 1],
                            mybir.AluOpType.mult,
                            mybir.AluOpType.add,
                        )

                    # PE-transpose [o, in'] -> [in', o]; in' = plane*NW + w
                    # fp8 transpose writes PSUM with element step 2.
                    oc = (ot % 4) * P
                    for kb in range(8):  # batches of 4 k-subtiles
                        tps = dq_psum.tile(
                            [P, 4, 2 * P], fp8, tag="tps", name=f"tps{ot}_{kb}"
                        )
                        for s in range(4):
                            it = kb * 4 + s      # ksub = plane*8 + wt
                            nc.tensor.transpose(
                                tps[:, s, ::2],
                                wd[:, it // NWP, ts(it % NWP, P)],
                                ident[:],
                            )
                        nc.any.tensor_copy(
                            wtq[qi][:, ts(kb, 4), ds(oc, P)], tps[:, :, ::2]
                        )

            # ---- Stage 2 emitter: one n-pair pass over all m-tiles ----
            def emit_cols(ns):
                for mb in range(N_MT // 4):
                    # fetch 4 m-tiles at once: 512B DMA lines
                    xt4 = xp.tile(
                        [P, KSUB, 512], fp8, tag="xt", name=f"xt{ns[0]}_{mb}"
                    )
                    nc.sync.dma_start(xt4[:], xv[:, :, ts(mb, 512)])
                    for mi in range(4):
                        m = 4 * mb + mi
                        msl = ts(m, P)
                        # both fp8 k-loops first, then both bf16 bias folds:
                        # one PE dtype-mode switch pair per m instead of two
                        pss = {}
                        for n in ns:
                            ps = mmp.tile(
                                [P, 512], f32, tag="ps", name=f"ps_{m}_{n}"
                            )
                            pss[n] = ps
                            for k in range(KSUB // 2):
                                kk = ts(k, 2)
                                nc.tensor.matmul(
                                    ps[:],
                                    xt4[:, kk, ts(mi, P)],
                                    wtq[n][:, kk, :],
                                    start=(k == 0),
                                    stop=False,
                                    perf_mode=mybir.MatmulPerfMode.DoubleRow,
                                )
                        for n in ns:
                            # exact affine remainder closes the group
                            nc.tensor.matmul(
                                pss[n][:],
                                tte_sb[:, msl],
                                be32_sb[:, ts(n, 512)],
                                start=False,
                                stop=True,
                                skip_group_check=True,
                            )
                        for n in ns:
                            ot_t = ev.tile(
                                [P, 512], f32, tag="ot", name=f"ot_{m}_{n}"
                            )
                            # eviction on the ACT engine keeps the DVE queue
                            # free for the next quarter's dequant
                            nc.scalar.activation(
                                ot_t[:],
                                pss[n][:],
                                mybir.ActivationFunctionType.Copy,
                                scale=1.0 / SC,
                            )
                            nc.sync.dma_start(y[msl, ts(n, 512)], ot_t[:])

            # interleaved emission: each matmul column starts as soon as its
            # W^T quarter exists; later quarters dequantize under the matmuls
            emit_quarter(0)
            emit_cols([0])
            emit_quarter(1)
            emit_cols([1])
            emit_quarter(2)
            emit_cols([2])
            emit_quarter(3)
            emit_cols([3])

    nc.compile()
    return nc


def _get_compiled():
    if "nc" not in _COMPILED:
        _COMPILED["nc"] = _build_nc()
    return _COMPILED["nc"]


def _marshal(input, w_packed, w_scale, w_bias, bias):
    import ml_dtypes

    x = np.ascontiguousarray(input, dtype=np.float32).reshape(BS, IN)
    # x^T rows permuted plane-major: row plane*1024 + w <- in = 4w + plane
    xt = x.T  # [IN, BS]
    xtp = np.ascontiguousarray(
        xt.reshape(NW, 4, BS).transpose(1, 0, 2).reshape(IN, BS)
    ).astype(ml_dtypes.float8_e4m3)

    # exact per-group input sums (+ ones row), padded to 128 partitions
    t = x.astype(np.float64).reshape(BS, G, IN // G).sum(axis=2)  # [BS, 16]
    tte = np.zeros((P, BS), dtype=ml_dtypes.bfloat16)
    tte[:G, :] = t.T.astype(ml_dtypes.bfloat16)
    tte[G, :] = np.ones(BS, dtype=ml_dtypes.bfloat16)

    s = w_scale.reshape(OUT, G).astype(np.float64)
    b = w_bias.reshape(OUT, G).astype(np.float64)
    be = SC * (7.5 * s + b)  # [OUT, 16]
    brow = SC * bias.reshape(OUT).astype(np.float64)

    wsc2 = (SC * s).astype(np.float32)
    wbi2 = (-7.5 * SC * s).astype(np.float32)

    # only the low u16 half of each int32 word carries nibbles (randint<2^16)
    wpk_u16 = np.ascontiguousarray(
        w_packed.reshape(OUT, NW).view(np.uint16)[:, 0::2]
    )

    id8 = np.eye(P, dtype=np.float32).astype(ml_dtypes.float8_e4m3)

    in_maps = []
    for c in range(NCORES):
        osl = slice(c * OSH, (c + 1) * OSH)
        be32 = np.zeros((P, OSH), dtype=ml_dtypes.bfloat16)
        be32[:G, :] = be[osl].T.astype(ml_dtypes.bfloat16)
        be32[G, :] = brow[osl].astype(ml_dtypes.bfloat16)
        in_maps.append(
            {
                "xtp": xtp,
                "wpk": np.ascontiguousarray(wpk_u16[osl]),
                "wsc": np.ascontiguousarray(wsc2[osl]),
                "wbi": np.ascontiguousarray(wbi2[osl]),
                "tte": tte,
                "id8": id8,
                "be32": be32,
            }
        )
    return in_maps


def kernel(input, w_packed, w_scale, w_bias, bias, _trace=False, _trace_kwargs=None):
    from concourse.bass_utils import run_bass_kernel_spmd

    nc = _get_compiled()
    in_maps = _marshal(input, w_packed, w_scale, w_bias, bias)
    res = run_bass_kernel_spmd(
        nc,
        in_maps,
        core_ids=list(range(NCORES)),
        trace=_trace,
        **(_trace_kwargs or {}),
    )
    ys = [res.results[c]["y"] for c in range(NCORES)]
    out = np.concatenate(ys, axis=1).reshape(B, S, OUT).astype(np.float32)
    if _trace:
        return out, res
    return out
